# revision 47
# baseline (speedup 1.0000x reference)
"""Trainium2 Bass kernel for CustomMultiHeadAttention with relative position
bias (Music-Transformer skew), causal mask.

Sharding: pure data-parallel over batch — B=8 batches, one per NeuronCore.
Weights and Er replicated; no collectives.

Per-core pipeline (v3):
  - Stages q -> v -> k: batched natural loads (ACT DMA queue), PE-transpose
    via fp16 identity (fast path), fp16 xT/wT; fp16 projections.
  - QEr strip generation for head-pairs is interleaved BETWEEN stages so the
    skew DRAM round trip (fp8e4m3) overlaps the projection phase.
  - Shear-read reconstructs Srel in natural [s, t] strip layout (fp8);
    causal diagonal blocks masked with -240 via affine_select (post-shear).
  - Per (t-tile): QK matmul (fp16) + transpose-via-identity matmuls
    accumulate Srel^T into the same 2-bank PSUM group; one fused
    Exp(scale=1/8) per t-tile writes fp16 A^T.
  - A^T @ V per s-half; row-sums via a ones row in v16; reciprocal + PE
    broadcast; normalize into fp16 attn_outT.
  - Output projection in natural layout (lhsT = attn_outT chunks), +bo,
    DMA out per 128-row strip.
"""

import numpy as np

import concourse.bass as bass
import concourse.tile as tile
from concourse import bacc, mybir
from concourse.bass import AP
from concourse.bass_utils import run_bass_kernel_spmd
from concourse.masks import make_identity

N_CORES = 8
B, S, D, H, DK = 8, 1024, 768, 12, 64
NT = S // 128          # 8 s-tiles
NI = D // 128          # 6 d-blocks
f32 = mybir.dt.float32
f16 = mybir.dt.float16
f8 = mybir.dt.float8e4
f32r = mybir.dt.float32r
NEG_FILL = -240.0      # e4m3-representable; exp((x-240)/8) == 0 for |x|<100

# packed-causal strip offsets: strip si (width 128*(si+1)) at column OFFS[si]
OFFS = [0]
for _si in range(NT):
    OFFS.append(OFFS[-1] + 128 * (_si + 1))
SREL_W = OFFS[-1]          # 4608
# A16T packed-causal: block ti (width 1024-128*ti) at col OT[ti]
OT = [0]
for _ti in range(NT):
    OT.append(OT[-1] + S - 128 * _ti)
AW = OT[-1]                # 4608


def build_nc():
    nc = bacc.Bacc("TRN2", target_bir_lowering=False, debug=False,
                   num_devices=N_CORES)

    Qb = nc.dram_tensor("Qb", [S, D], f32, kind="ExternalInput")
    Kb = nc.dram_tensor("Kb", [S, D], f32, kind="ExternalInput")
    Vb = nc.dram_tensor("Vb", [S, D], f32, kind="ExternalInput")
    Wq = nc.dram_tensor("Wq", [D, D], f32, kind="ExternalInput")
    Wk = nc.dram_tensor("Wk", [D, D], f32, kind="ExternalInput")
    Wv = nc.dram_tensor("Wv", [D, D], f32, kind="ExternalInput")
    Wo = nc.dram_tensor("Wo", [D, D], f32, kind="ExternalInput")
    bq = nc.dram_tensor("bq", [D], f32, kind="ExternalInput")
    bk = nc.dram_tensor("bk", [D], f32, kind="ExternalInput")
    bv = nc.dram_tensor("bv", [D], f32, kind="ExternalInput")
    bo = nc.dram_tensor("bo", [D], f32, kind="ExternalInput")
    Er = nc.dram_tensor("Er", [S, DK], f32, kind="ExternalInput")
    out = nc.dram_tensor("out", [S, D], f32, kind="ExternalOutput")

    # DRAM skew scratch: 3 pair-regions (round-robin) of 2x[128, SREL_W] fp8
    qer_dram = nc.dram_tensor("qer_scratch", [6 * 128 * SREL_W], f16)

    tensors = dict(Qb=Qb, Kb=Kb, Vb=Vb, Wq=Wq, Wk=Wk, Wv=Wv, Wo=Wo,
                   bq=bq, bk=bk, bv=bv, bo=bo, Er=Er, out=out,
                   qer_dram=qer_dram)
    with tile.TileContext(nc) as tc:
        _build_body(nc, tc, tensors)
    nc.compile()
    return nc


class _RR:
    """Round-robin eviction across DVE / ACT / Pool copy paths."""

    _PAT = ("v", "a")

    def __init__(self, nc):
        self.nc = nc
        self.i = 0

    def copy(self, dst, src):
        eng = self._PAT[self.i % len(self._PAT)]
        self.i += 1
        if eng == "v":
            self.nc.vector.tensor_copy(dst, src)
        elif eng == "a":
            self.nc.scalar.copy(dst, src)
        else:
            self.nc.gpsimd.tensor_copy(dst, src)


def _stream_transpose(nc, ps, rr, stg, src_dram, nrow, ncol, dst_tiles,
                      ident16, tag):
    """Stream row-pair loads (ACT DMA queue) through a rotating 2-buf
    staging tile; PE-transpose (fp16 identity) into dst_tiles[ct]
    (f16, [128, nrow]) with [128, 256] grouped PSUM evictions."""
    nrt = nrow // 128
    nct = ncol // 128
    half = nrt // 2
    src3 = src_dram.ap().rearrange("(b p) d -> p b d", p=128)
    for c0 in range(0, nrt, half):
        nat = stg.tile([128, half, ncol], f32, tag=tag,
                       name=f"nat_{tag}{c0}", bufs=2)
        nc.sync.dma_start(out=nat[:], in_=src3[:, c0:c0 + half, :])
        for ct in range(nct):
            p = ps.tile([128, 512], f32, tag="ps_t", bufs=4)
            for k in range(half):
                nc.tensor.matmul(
                    p[:, k * 128:(k + 1) * 128].bitcast(f32r),
                    nat[:, k, ct * 128:(ct + 1) * 128].bitcast(f32r),
                    ident32[:].bitcast(f32r),
                    is_transpose=True,
                    start=(k == 0), stop=(k == half - 1),
                )
            rr.copy(dst_tiles[ct][:, c0 * 128:(c0 + half) * 128],
                    p[:, :half * 128])


def _build_body(nc, tc, t):
    Qb, Kb, Vb = t["Qb"], t["Kb"], t["Vb"]
    Wq, Wk, Wv, Wo = t["Wq"], t["Wk"], t["Wv"], t["Wo"]
    bq, bk, bv, bo = t["bq"], t["bk"], t["bv"], t["bo"]
    Er, out, qer_dram = t["Er"], t["out"], t["qer_dram"]

    rr = _RR(nc)

    from contextlib import ExitStack
    with ExitStack() as ctx:
        persist = ctx.enter_context(tc.tile_pool(name="persist", bufs=1))
        P = {}
        ps_misc_cm = tc.tile_pool(name="ps_misc", bufs=2, space="PSUM")
        ps_misc = ps_misc_cm.__enter__()

        ident16 = persist.tile([128, 128], f16, tag="ident16")
        make_identity(nc, ident16[:])
        ident32 = persist.tile([128, 128], f32, tag="ident32")
        make_identity(nc, ident32[:])

        # ---- biases ----
        bq_col = persist.tile([128, NI], f32, tag="bq_col")
        bk_col = persist.tile([128, NI], f32, tag="bk_col")
        for jt in range(NI):
            nc.sync.dma_start(out=bq_col[:, jt:jt + 1],
                              in_=bq.ap()[jt * 128:(jt + 1) * 128].unsqueeze(1))
            nc.sync.dma_start(out=bk_col[:, jt:jt + 1],
                              in_=bk.ap()[jt * 128:(jt + 1) * 128].unsqueeze(1))
        bv_row = persist.tile([128, D], f32, tag="bv_row")
        nc.sync.dma_start(out=bv_row[:],
                          in_=AP(tensor=bv, offset=0, ap=[[0, 128], [1, D]]))
        bo_row = persist.tile([128, D], f32, tag="bo_row")
        nc.sync.dma_start(out=bo_row[:],
                          in_=AP(tensor=bo, offset=0, ap=[[0, 128], [1, D]]))

        # ---- ErT duplicated into both partition halves [128, 1024] f16 ----
        erT16 = persist.tile([128, S], f16, tag="erT16")
        with tc.tile_pool(name="er_stage", bufs=1) as erp:
            er_nat = erp.tile([128, NT, DK], f32, tag="er_nat")
            nc.sync.dma_start(
                out=er_nat[:],
                in_=Er.ap().rearrange("(b p) d -> p b d", p=128))
            for et in range(NT):
                p = ps_misc.tile([128, 512], f32, tag="ps_t", bufs=4)
                nc.tensor.matmul(p[:DK, :128], er_nat[:, et, :],
                                 ident32[:],
                                 is_transpose=True, start=True, stop=True)
                nc.scalar.copy(erT16[0:DK, et * 128:(et + 1) * 128],
                               p[:DK, :128])
                nc.scalar.copy(erT16[DK:128, et * 128:(et + 1) * 128],
                               p[:DK, :128])

        # ---- persistent projection outputs ----
        qT = [persist.tile([128, S], f16, tag=f"qT{i}", name=f"qT{i}")
              for i in range(NI)]
        kT = [persist.tile([128, S], f16, tag=f"kT{i}", name=f"kT{i}")
              for i in range(NI)]
        v16 = [persist.tile([128, H * 65], f16, tag=f"v16{i}", name=f"v16{i}")
               for i in range(NT)]
        woT = [persist.tile([128, D], f16, tag=f"woT{i}", name=f"woT{i}")
               for i in range(NI)]
        attn_outT = [persist.tile([128, S], f16, tag=f"attn_outT{i}",
                                  name=f"attn_outT{i}") for i in range(NI)]
        for tt in range(NT):
            nc.vector.memset(v16[tt][:], 1.0)

        # ---- per-head-pair QEr emission (interleaved between stages) ----
        hp_state = {}

        def emit_qer(hp):
            reg0 = (hp % 3) * 2 * 128 * SREL_W
            work, ps_e = P["work"], P["ps_e"]
            srel_out2 = work.tile([128, 2, SREL_W], f16, tag="srel_out",
                                  name=f"srel_out_hp{hp}", bufs=2)
            srel16_2 = work.tile([128, 2, SREL_W], f16, tag="srel16",
                                 name=f"srel16_hp{hp}", bufs=2)
            hp_state[hp] = srel16_2
            jb = hp
            for si in range(NT):
                Wcw = 128 * (si + 1)
                e0 = S - Wcw
                for cs in range(0, Wcw, 512):
                    w = min(512, Wcw - cs)
                    for idx in range(2):
                        jr = 64 * idx
                        p = ps_e.tile([128, 512], f32, tag="pse", bufs=2)
                        nc.tensor.matmul(
                            p[:, :w],
                            qT[jb][jr:jr + 64, si * 128:(si + 1) * 128],
                            erT16[jr:jr + 64, e0 + cs:e0 + cs + w],
                            start=True, stop=True)
                        dstp = srel_out2[:, idx, OFFS[si] + cs:
                                         OFFS[si] + cs + w]
                        if (si + idx) % 2 == 0:
                            nc.vector.tensor_copy(dstp, p[:, :w])
                        else:
                            nc.scalar.copy(dstp, p[:, :w])
            dst = AP(tensor=qer_dram, offset=reg0,
                     ap=[[SREL_W, 128], [128 * SREL_W, 2], [1, SREL_W]])
            nc.sync.dma_start(out=dst, in_=srel_out2[:])
            # shear reads (fp8), both heads per DMA
            for si in range(NT):
                Wcw = 128 * (si + 1)
                skew = AP(tensor=qer_dram,
                          offset=reg0 + OFFS[si] + 127,
                          ap=[[SREL_W - 1, 128], [128 * SREL_W, 2], [1, Wcw]])
                nc.sync.dma_start(
                    out=srel16_2[:, :, OFFS[si]:OFFS[si] + Wcw],
                    in_=skew)
            for si in range(NT):
                Wcw = 128 * (si + 1)
                for idx in range(2):
                    dblk = srel16_2[:, idx, OFFS[si] + Wcw - 128:
                                    OFFS[si] + Wcw]
                    nc.gpsimd.affine_select(
                        out=dblk, in_=dblk,
                        pattern=[[-1, 128]], channel_multiplier=1,
                        compare_op=mybir.AluOpType.is_ge,
                        base=0, fill=NEG_FILL)

        # ---- stages: q -> v -> k (+ wo), with QEr interleaved ----
        wstage = {}

        def stage(tag, X, Wt):
            stg_cm = tc.tile_pool(name=f"stage_{tag}", bufs=1)
            stg = stg_cm.__enter__()
            wT = [stg.tile([128, D], f16, tag=f"wT{tag}{i}",
                           name=f"wT{tag}{i}") for i in range(NI)]
            xT = [stg.tile([128, S], f16, tag=f"xT{tag}{i}",
                           name=f"xT{tag}{i}") for i in range(NI)]
            _stream_transpose(nc, ps_misc, rr, stg, Wt, D, D, wT, ident16,
                              f"natW{tag}")
            _stream_transpose(nc, ps_misc, rr, stg, X, S, D, xT, ident16,
                              f"natX{tag}")
            wstage[tag] = (stg_cm, wT, xT)

        def proj_qk(tag, bias_col, xT_out, use_dve):
            _, wT, xT = wstage[tag]
            for jt in range(NI):
                for sh in range(2):
                    p = ps_misc.tile([128, 512], f32, tag="ps_p", bufs=4)
                    for ib in range(NI):
                        nc.tensor.matmul(
                            p[:],
                            wT[ib][:, jt * 128:(jt + 1) * 128],
                            xT[ib][:, sh * 512:(sh + 1) * 512],
                            start=(ib == 0), stop=(ib == NI - 1),
                        )
                    if use_dve:
                        nc.vector.tensor_scalar_add(
                            xT_out[jt][:, sh * 512:(sh + 1) * 512],
                            p[:], bias_col[:, jt:jt + 1])
                    else:
                        nc.scalar.activation(
                            xT_out[jt][:, sh * 512:(sh + 1) * 512], p[:],
                            mybir.ActivationFunctionType.Identity,
                            bias=bias_col[:, jt:jt + 1])
            wstage[tag][0].__exit__(None, None, None)

        def proj_v():
            _, wT, xT = wstage["v"]
            for tt in range(NT):
                for js, w in ((0, 512), (512, 256)):
                    p = ps_misc.tile([128, 512], f32, tag="ps_p", bufs=4)
                    for ib in range(NI):
                        nc.tensor.matmul(
                            p[:, :w],
                            xT[ib][:, tt * 128:(tt + 1) * 128],
                            wT[ib][:, js:js + w],
                            start=(ib == 0), stop=(ib == NI - 1),
                        )
                    hh0 = js // 64
                    nh = w // 64
                    dst3 = v16[tt][:, :].rearrange(
                        "p (a b) -> p a b", b=65)[:, hh0:hh0 + nh, 0:64]
                    nc.vector.tensor_add(
                        dst3, p[:, :w].rearrange("p (a b) -> p a b", b=64),
                        bv_row[:, js:js + w].rearrange(
                            "p (a b) -> p a b", b=64))
            wstage["v"][0].__exit__(None, None, None)

        stage("q", Qb, Wq)
        proj_qk("q", bq_col, qT, use_dve=True)
        stage("k", Kb, Wk)
        proj_qk("k", bk_col, kT, use_dve=False)
        stage("v", Vb, Wv)
        proj_v()
        with tc.tile_pool(name="stage_wo", bufs=1) as stgo:
            _stream_transpose(nc, ps_misc, rr, stgo, Wo, D, D, woT, ident16,
                              "natWo")

        ps_misc_cm.__exit__(None, None, None)

        # ---- attention QK/exp/AV per head-pair ----
        with tc.tile_pool(name="work", bufs=1) as work, \
             tc.tile_pool(name="ps_e", bufs=2, space="PSUM") as ps_e, \
             tc.tile_pool(name="ps_qk", bufs=2, space="PSUM") as ps_qk, \
             tc.tile_pool(name="ps_av", bufs=1, space="PSUM") as ps_av:
            P["work"], P["ps_e"] = work, ps_e
            ones1f = work.tile([1, 64], f32, tag="ones1f", bufs=1)
            nc.vector.memset(ones1f[:], 1.0)
            ones1 = work.tile([1, 64], f32r, tag="ones1", bufs=1)
            nc.vector.tensor_copy(ones1[:], ones1f[:])
            for hp in range(H // 2):
                jb = hp
                emit_qer(hp)
                srel16_2 = hp_state[hp]
                ctxs = [dict(h=2 * hp + idx, jr=64 * idx, idx=idx)
                        for idx in range(2)]
                for c in ctxs:
                    c["A16T"] = work.tile(
                        [128, AW], f16, tag=f"A16T{c['idx']}",
                        name=f"A16T{c['h']}", bufs=2)
                for ti in range(NT):
                    s0 = 128 * ti
                    w = S - s0
                    for c in ctxs:
                        jr = c["jr"]
                        pqk = ps_qk.tile([128, 1024], f32, tag="qk",
                                         bufs=2, name=f"pqk{c['h']}_{ti}")
                        for sub in range(0, w, 512):
                            sw = min(512, w - sub)
                            nc.tensor.matmul(
                                pqk[:, sub:sub + sw],
                                kT[jb][jr:jr + 64, ti * 128:(ti + 1) * 128],
                                qT[jb][jr:jr + 64, s0 + sub:s0 + sub + sw],
                                start=True, stop=False)
                            for k in range(sw // 128):
                                sic = ti + (sub + k * 128) // 128
                                nc.tensor.matmul(
                                    pqk[:, sub + k * 128:sub + (k + 1) * 128],
                                    srel16_2[:, c["idx"],
                                             OFFS[sic] + 128 * ti:
                                             OFFS[sic] + 128 * ti + 128],
                                    ident16[:],
                                    start=False, stop=(k == sw // 128 - 1))
                        nc.scalar.activation(
                            c["A16T"][:, OT[ti]:OT[ti] + w],
                            pqk[:, :w],
                            mybir.ActivationFunctionType.Exp, scale=0.125)

                # --- AV + normalize per head, per s-half ---
                for c in ctxs:
                    h, jr = c["h"], c["jr"]
                    odd_tmp = None
                    if h % 2 == 1:
                        odd_tmp = work.tile([64, S], f16, tag="odd_tmp",
                                            name=f"ot{h}", bufs=2)
                    for half in range(2):
                        sh0 = half * 512
                        pav = ps_av.tile([65, 512], f32, tag="av",
                                         name=f"pav{h}_{half}", bufs=2)
                        mms = []
                        for ti in range(NT):
                            lo = max(sh0, 128 * ti)
                            hi = sh0 + 512
                            if lo >= hi:
                                continue
                            mms.append((ti, lo, hi))
                        for n, (ti, lo, hi) in enumerate(mms):
                            v16s = v16[ti][:, h * 65:(h + 1) * 65]
                            nc.tensor.matmul(
                                pav[:, lo - sh0:hi - sh0], v16s,
                                c["A16T"][:, OT[ti] + lo - 128 * ti:
                                          OT[ti] + hi - 128 * ti],
                                start=(n == 0), stop=(n == len(mms) - 1))
                        rZ_row = work.tile([1, 512], f32r, tag="rZ_row",
                                           name=f"rZ{h}_{half}", bufs=2)
                        with nc.allow_low_precision(reason="f32r==f32 bits"):
                            nc.vector.reciprocal(rZ_row[:], pav[64:65, :])
                        przi = ps_qk.tile([64, 512], f32, tag="qk",
                                          name=f"prz{h}_{half}", bufs=2)
                        nc.tensor.matmul(przi[:], ones1[:], rZ_row[:],
                                         start=True, stop=True)
                        rzb = work.tile([64, 512], f32, tag="rzb_sb", bufs=2,
                                        name=f"rzb{h}_{half}")
                        if half == 0:
                            nc.vector.tensor_copy(rzb[:], przi[:])
                        else:
                            nc.scalar.copy(rzb[:], przi[:])
                        if h % 2 == 0:
                            nc.vector.tensor_mul(
                                attn_outT[jb][0:64, sh0:sh0 + 512],
                                pav[0:64, :], rzb[:])
                        else:
                            nc.vector.tensor_mul(odd_tmp[:, sh0:sh0 + 512],
                                                 pav[0:64, :], rzb[:])
                    if h % 2 == 1:
                        nc.sync.dma_start(out=attn_outT[jb][64:128, :],
                                            in_=odd_tmp[:])

        # ---- output projection (natural layout, direct) ----
        with tc.tile_pool(name="ps_o", bufs=2, space="PSUM") as ps_o, \
             tc.tile_pool(name="stage_o", bufs=2) as stg:
            for sc in range(NT):
                po1 = ps_o.tile([128, 512], f32, tag="o1")
                po2 = ps_o.tile([128, 256], f32, tag="o2")
                for ib in range(NI):
                    lhs = attn_outT[ib][:, sc * 128:(sc + 1) * 128]
                    nc.tensor.matmul(po1[:], lhs, woT[ib][:, 0:512],
                                     start=(ib == 0), stop=(ib == NI - 1))
                    nc.tensor.matmul(po2[:], lhs, woT[ib][:, 512:768],
                                     start=(ib == 0), stop=(ib == NI - 1))
                outs = stg.tile([128, D], f32, tag="out_strip")
                nc.vector.tensor_add(outs[:, 0:512], po1[:],
                                     bo_row[:, 0:512])
                nc.vector.tensor_add(outs[:, 512:768], po2[:],
                                     bo_row[:, 512:768])
                nc.sync.dma_start(out=out.ap()[sc * 128:(sc + 1) * 128, :],
                                  in_=outs[:])


_NC = None


def kernel(**inputs):
    global _NC
    if _NC is None:
        _NC = build_nc()
    Q = np.ascontiguousarray(np.asarray(inputs["Q"], dtype=np.float32))
    K = np.ascontiguousarray(np.asarray(inputs["K"], dtype=np.float32))
    V = np.ascontiguousarray(np.asarray(inputs["V"], dtype=np.float32))
    shared = {
        name: np.ascontiguousarray(np.asarray(inputs[name], dtype=np.float32))
        for name in ("Wq", "Wk", "Wv", "Wo", "bq", "bk", "bv", "bo", "Er")
    }
    in_maps = [
        {"Qb": Q[c], "Kb": K[c], "Vb": V[c], **shared} for c in range(N_CORES)
    ]
    global _last_in_maps
    _last_in_maps = in_maps
    res = run_bass_kernel_spmd(_NC, in_maps, list(range(N_CORES)))
    return np.stack([res.results[c]["out"] for c in range(N_CORES)], axis=0)


# revision 53
# speedup vs baseline: 1.0153x; 1.0153x over previous
"""Trainium2 Bass kernel for CustomMultiHeadAttention with relative position
bias (Music-Transformer skew), causal mask.

Sharding: pure data-parallel over batch — B=8 batches, one per NeuronCore.
Weights and Er replicated; no collectives.

Per-core pipeline (v3):
  - Stages q -> v -> k: batched natural loads (ACT DMA queue), PE-transpose
    via fp16 identity (fast path), fp16 xT/wT; fp16 projections.
  - QEr strip generation for head-pairs is interleaved BETWEEN stages so the
    skew DRAM round trip (fp8e4m3) overlaps the projection phase.
  - Shear-read reconstructs Srel in natural [s, t] strip layout (fp8);
    causal diagonal blocks masked with -240 via affine_select (post-shear).
  - Per (t-tile): QK matmul (fp16) + transpose-via-identity matmuls
    accumulate Srel^T into the same 2-bank PSUM group; one fused
    Exp(scale=1/8) per t-tile writes fp16 A^T.
  - A^T @ V per s-half; row-sums via a ones row in v16; reciprocal + PE
    broadcast; normalize into fp16 attn_outT.
  - Output projection in natural layout (lhsT = attn_outT chunks), +bo,
    DMA out per 128-row strip.
"""

import numpy as np

import concourse.bass as bass
import concourse.tile as tile
from concourse import bacc, mybir
from concourse.bass import AP
from concourse.bass_utils import run_bass_kernel_spmd
from concourse.masks import make_identity

N_CORES = 8
B, S, D, H, DK = 8, 1024, 768, 12, 64
NT = S // 128          # 8 s-tiles
NI = D // 128          # 6 d-blocks
f32 = mybir.dt.float32
f16 = mybir.dt.float16
f8 = mybir.dt.float8e4
f32r = mybir.dt.float32r
NEG_FILL = -240.0      # e4m3-representable; exp((x-240)/8) == 0 for |x|<100

# packed-causal strip offsets: strip si (width 128*(si+1)) at column OFFS[si]
OFFS = [0]
for _si in range(NT):
    OFFS.append(OFFS[-1] + 128 * (_si + 1))
SREL_W = OFFS[-1]          # 4608
# A16T packed-causal: block ti (width 1024-128*ti) at col OT[ti]
OT = [0]
for _ti in range(NT):
    OT.append(OT[-1] + S - 128 * _ti)
AW = OT[-1]                # 4608


def build_nc():
    nc = bacc.Bacc("TRN2", target_bir_lowering=False, debug=False,
                   num_devices=N_CORES)

    Qb = nc.dram_tensor("Qb", [S, D], f32, kind="ExternalInput")
    Kb = nc.dram_tensor("Kb", [S, D], f32, kind="ExternalInput")
    Vb = nc.dram_tensor("Vb", [S, D], f32, kind="ExternalInput")
    Wq = nc.dram_tensor("Wq", [D, D], f32, kind="ExternalInput")
    Wk = nc.dram_tensor("Wk", [D, D], f32, kind="ExternalInput")
    Wv = nc.dram_tensor("Wv", [D, D], f32, kind="ExternalInput")
    Wo = nc.dram_tensor("Wo", [D, D], f32, kind="ExternalInput")
    bq = nc.dram_tensor("bq", [D], f32, kind="ExternalInput")
    bk = nc.dram_tensor("bk", [D], f32, kind="ExternalInput")
    bv = nc.dram_tensor("bv", [D], f32, kind="ExternalInput")
    bo = nc.dram_tensor("bo", [D], f32, kind="ExternalInput")
    Er = nc.dram_tensor("Er", [S, DK], f32, kind="ExternalInput")
    out = nc.dram_tensor("out", [S, D], f32, kind="ExternalOutput")

    # DRAM skew scratch: 3 pair-regions (round-robin) of 2x[128, SREL_W] fp8
    qer_dram = nc.dram_tensor("qer_scratch", [6 * 128 * SREL_W], f16)

    tensors = dict(Qb=Qb, Kb=Kb, Vb=Vb, Wq=Wq, Wk=Wk, Wv=Wv, Wo=Wo,
                   bq=bq, bk=bk, bv=bv, bo=bo, Er=Er, out=out,
                   qer_dram=qer_dram)
    with tile.TileContext(nc) as tc:
        _build_body(nc, tc, tensors)
    nc.compile()
    return nc


class _RR:
    """Round-robin eviction across DVE / ACT / Pool copy paths."""

    _PAT = ("v", "a")

    def __init__(self, nc):
        self.nc = nc
        self.i = 0

    def copy(self, dst, src):
        eng = self._PAT[self.i % len(self._PAT)]
        self.i += 1
        if eng == "v":
            self.nc.vector.tensor_copy(dst, src)
        elif eng == "a":
            self.nc.scalar.copy(dst, src)
        else:
            self.nc.gpsimd.tensor_copy(dst, src)


def _stream_transpose(nc, ps, rr, stg, src_dram, nrow, ncol, dst_tiles,
                      ident16, tag):
    """Stream row-pair loads (ACT DMA queue) through a rotating 2-buf
    staging tile; PE-transpose (fp16 identity) into dst_tiles[ct]
    (f16, [128, nrow]) with [128, 256] grouped PSUM evictions."""
    nrt = nrow // 128
    nct = ncol // 128
    half = nrt // 2
    src3 = src_dram.ap().rearrange("(b p) d -> p b d", p=128)
    for c0 in range(0, nrt, half):
        nat = stg.tile([128, half, ncol], f32, tag=tag,
                       name=f"nat_{tag}{c0}", bufs=2)
        nc.sync.dma_start(out=nat[:], in_=src3[:, c0:c0 + half, :])
        for ct in range(nct):
            p = ps.tile([128, 512], f32, tag="ps_t", bufs=4)
            for k in range(half):
                nc.tensor.matmul(
                    p[:, k * 128:(k + 1) * 128].bitcast(f32r),
                    nat[:, k, ct * 128:(ct + 1) * 128].bitcast(f32r),
                    ident32[:].bitcast(f32r),
                    is_transpose=True,
                    start=(k == 0), stop=(k == half - 1),
                )
            rr.copy(dst_tiles[ct][:, c0 * 128:(c0 + half) * 128],
                    p[:, :half * 128])


def _build_body(nc, tc, t):
    Qb, Kb, Vb = t["Qb"], t["Kb"], t["Vb"]
    Wq, Wk, Wv, Wo = t["Wq"], t["Wk"], t["Wv"], t["Wo"]
    bq, bk, bv, bo = t["bq"], t["bk"], t["bv"], t["bo"]
    Er, out, qer_dram = t["Er"], t["out"], t["qer_dram"]

    rr = _RR(nc)

    from contextlib import ExitStack
    with ExitStack() as ctx:
        persist = ctx.enter_context(tc.tile_pool(name="persist", bufs=1))
        P = {}
        ps_misc_cm = tc.tile_pool(name="ps_misc", bufs=2, space="PSUM")
        ps_misc = ps_misc_cm.__enter__()

        ident16 = persist.tile([128, 128], f16, tag="ident16")
        make_identity(nc, ident16[:])
        ident32 = persist.tile([128, 128], f32, tag="ident32")
        make_identity(nc, ident32[:])

        # ---- biases ----
        bq_col = persist.tile([128, NI], f32, tag="bq_col")
        bk_col = persist.tile([128, NI], f32, tag="bk_col")
        for jt in range(NI):
            nc.sync.dma_start(out=bq_col[:, jt:jt + 1],
                              in_=bq.ap()[jt * 128:(jt + 1) * 128].unsqueeze(1))
            nc.sync.dma_start(out=bk_col[:, jt:jt + 1],
                              in_=bk.ap()[jt * 128:(jt + 1) * 128].unsqueeze(1))
        bv_row = persist.tile([128, D], f32, tag="bv_row")
        nc.sync.dma_start(out=bv_row[:],
                          in_=AP(tensor=bv, offset=0, ap=[[0, 128], [1, D]]))
        bo_row = persist.tile([128, D], f32, tag="bo_row")
        nc.sync.dma_start(out=bo_row[:],
                          in_=AP(tensor=bo, offset=0, ap=[[0, 128], [1, D]]))

        # ---- ErT duplicated into both partition halves [128, 1024] f16 ----
        erT16 = persist.tile([128, S], f16, tag="erT16")
        with tc.tile_pool(name="er_stage", bufs=1) as erp:
            er_nat = erp.tile([128, NT, DK], f32, tag="er_nat")
            nc.sync.dma_start(
                out=er_nat[:],
                in_=Er.ap().rearrange("(b p) d -> p b d", p=128))
            for et in range(NT):
                p = ps_misc.tile([128, 512], f32, tag="ps_p", bufs=4)
                nc.tensor.matmul(p[:DK, :128], er_nat[:, et, :],
                                 ident32[:],
                                 is_transpose=True, start=True, stop=True)
                nc.scalar.copy(erT16[0:DK, et * 128:(et + 1) * 128],
                               p[:DK, :128])
                nc.scalar.copy(erT16[DK:128, et * 128:(et + 1) * 128],
                               p[:DK, :128])

        # ---- persistent projection outputs ----
        qT = [persist.tile([128, S], f16, tag=f"qT{i}", name=f"qT{i}")
              for i in range(NI)]
        kT = [persist.tile([128, S], f16, tag=f"kT{i}", name=f"kT{i}")
              for i in range(NI)]
        v16 = [persist.tile([128, H * 65], f16, tag=f"v16{i}", name=f"v16{i}")
               for i in range(NT)]
        woT = [persist.tile([128, D], f16, tag=f"woT{i}", name=f"woT{i}")
               for i in range(NI)]
        attn_outT = [persist.tile([128, S], f16, tag=f"attn_outT{i}",
                                  name=f"attn_outT{i}") for i in range(NI)]
        for tt in range(NT):
            nc.gpsimd.memset(v16[tt][:], 1.0)

        # ---- per-head-pair QEr emission (interleaved between stages) ----
        hp_state = {}

        def emit_qer(hp):
            reg0 = (hp % 3) * 2 * 128 * SREL_W
            work, ps_e = P["work"], P["ps_e"]
            srel_out2 = work.tile([128, 2, SREL_W], f16, tag="srel_out",
                                  name=f"srel_out_hp{hp}", bufs=2)
            srel16_2 = work.tile([128, 2, SREL_W], f16, tag="srel16",
                                 name=f"srel16_hp{hp}", bufs=2)
            hp_state[hp] = srel16_2
            jb = hp
            for si in range(NT):
                Wcw = 128 * (si + 1)
                e0 = S - Wcw
                for cs in range(0, Wcw, 512):
                    w = min(512, Wcw - cs)
                    for idx in range(2):
                        jr = 64 * idx
                        p = ps_e.tile([128, 512], f32, tag="pse", bufs=2)
                        nc.tensor.matmul(
                            p[:, :w],
                            qT[jb][jr:jr + 64, si * 128:(si + 1) * 128],
                            erT16[jr:jr + 64, e0 + cs:e0 + cs + w],
                            start=True, stop=True)
                        dstp = srel_out2[:, idx, OFFS[si] + cs:
                                         OFFS[si] + cs + w]
                        if (si + idx) % 4 != 3:
                            nc.vector.tensor_copy(dstp, p[:, :w])
                        else:
                            nc.scalar.copy(dstp, p[:, :w])
            dst = AP(tensor=qer_dram, offset=reg0,
                     ap=[[SREL_W, 128], [128 * SREL_W, 2], [1, SREL_W]])
            nc.sync.dma_start(out=dst, in_=srel_out2[:])
            # shear reads (fp8), both heads per DMA
            for si in range(NT):
                Wcw = 128 * (si + 1)
                skew = AP(tensor=qer_dram,
                          offset=reg0 + OFFS[si] + 127,
                          ap=[[SREL_W - 1, 128], [128 * SREL_W, 2], [1, Wcw]])
                nc.sync.dma_start(
                    out=srel16_2[:, :, OFFS[si]:OFFS[si] + Wcw],
                    in_=skew)
            for si in range(NT):
                Wcw = 128 * (si + 1)
                for idx in range(2):
                    dblk = srel16_2[:, idx, OFFS[si] + Wcw - 128:
                                    OFFS[si] + Wcw]
                    nc.gpsimd.affine_select(
                        out=dblk, in_=dblk,
                        pattern=[[-1, 128]], channel_multiplier=1,
                        compare_op=mybir.AluOpType.is_ge,
                        base=0, fill=NEG_FILL)

        # ---- stages: q -> v -> k (+ wo), with QEr interleaved ----
        wstage = {}

        def stage(tag, X, Wt):
            stg_cm = tc.tile_pool(name=f"stage_{tag}", bufs=1)
            stg = stg_cm.__enter__()
            wT = [stg.tile([128, D], f16, tag=f"wT{tag}{i}",
                           name=f"wT{tag}{i}") for i in range(NI)]
            xT = [stg.tile([128, S], f16, tag=f"xT{tag}{i}",
                           name=f"xT{tag}{i}") for i in range(NI)]
            _stream_transpose(nc, ps_misc, rr, stg, Wt, D, D, wT, ident16,
                              f"natW{tag}")
            _stream_transpose(nc, ps_misc, rr, stg, X, S, D, xT, ident16,
                              f"natX{tag}")
            wstage[tag] = (stg_cm, wT, xT)

        def proj_qk(tag, bias_col, xT_out, use_dve):
            _, wT, xT = wstage[tag]
            for jt in range(NI):
                for sh in range(2):
                    p = ps_misc.tile([128, 512], f32, tag="ps_p", bufs=4)
                    for ib in range(NI):
                        nc.tensor.matmul(
                            p[:],
                            wT[ib][:, jt * 128:(jt + 1) * 128],
                            xT[ib][:, sh * 512:(sh + 1) * 512],
                            start=(ib == 0), stop=(ib == NI - 1),
                        )
                    if use_dve:
                        nc.vector.tensor_scalar_add(
                            xT_out[jt][:, sh * 512:(sh + 1) * 512],
                            p[:], bias_col[:, jt:jt + 1])
                    else:
                        nc.scalar.activation(
                            xT_out[jt][:, sh * 512:(sh + 1) * 512], p[:],
                            mybir.ActivationFunctionType.Identity,
                            bias=bias_col[:, jt:jt + 1])
            wstage[tag][0].__exit__(None, None, None)

        def proj_v():
            _, wT, xT = wstage["v"]
            for tt in range(NT):
                for js, w in ((0, 512), (512, 256)):
                    p = ps_misc.tile([128, 512], f32, tag="ps_p", bufs=4)
                    for ib in range(NI):
                        nc.tensor.matmul(
                            p[:, :w],
                            xT[ib][:, tt * 128:(tt + 1) * 128],
                            wT[ib][:, js:js + w],
                            start=(ib == 0), stop=(ib == NI - 1),
                        )
                    hh0 = js // 64
                    nh = w // 64
                    dst3 = v16[tt][:, :].rearrange(
                        "p (a b) -> p a b", b=65)[:, hh0:hh0 + nh, 0:64]
                    nc.vector.tensor_add(
                        dst3, p[:, :w].rearrange("p (a b) -> p a b", b=64),
                        bv_row[:, js:js + w].rearrange(
                            "p (a b) -> p a b", b=64))
            wstage["v"][0].__exit__(None, None, None)

        stage("q", Qb, Wq)
        proj_qk("q", bq_col, qT, use_dve=True)
        stage("k", Kb, Wk)
        proj_qk("k", bk_col, kT, use_dve=False)
        stage("v", Vb, Wv)
        proj_v()
        with tc.tile_pool(name="stage_wo", bufs=1) as stgo:
            _stream_transpose(nc, ps_misc, rr, stgo, Wo, D, D, woT, ident16,
                              "natWo")

        ps_misc_cm.__exit__(None, None, None)

        # ---- attention QK/exp/AV per head-pair ----
        with tc.tile_pool(name="work", bufs=1) as work, \
             tc.tile_pool(name="ps_e", bufs=2, space="PSUM") as ps_e, \
             tc.tile_pool(name="ps_qk", bufs=2, space="PSUM") as ps_qk, \
             tc.tile_pool(name="ps_av", bufs=1, space="PSUM") as ps_av:
            P["work"], P["ps_e"] = work, ps_e
            ones1f = work.tile([1, 64], f32, tag="ones1f", bufs=1)
            nc.vector.memset(ones1f[:], 1.0)
            ones1 = work.tile([1, 64], f32r, tag="ones1", bufs=1)
            nc.vector.tensor_copy(ones1[:], ones1f[:])
            for hp in range(H // 2):
                jb = hp
                emit_qer(hp)
                srel16_2 = hp_state[hp]
                ctxs = [dict(h=2 * hp + idx, jr=64 * idx, idx=idx)
                        for idx in range(2)]
                for c in ctxs:
                    c["A16T"] = work.tile(
                        [128, AW], f16, tag=f"A16T{c['idx']}",
                        name=f"A16T{c['h']}", bufs=2)
                for ti in range(NT):
                    s0 = 128 * ti
                    w = S - s0
                    for c in ctxs:
                        jr = c["jr"]
                        pqk = ps_qk.tile([128, 1024], f32, tag="qk",
                                         bufs=2, name=f"pqk{c['h']}_{ti}")
                        for sub in range(0, w, 512):
                            sw = min(512, w - sub)
                            nc.tensor.matmul(
                                pqk[:, sub:sub + sw],
                                kT[jb][jr:jr + 64, ti * 128:(ti + 1) * 128],
                                qT[jb][jr:jr + 64, s0 + sub:s0 + sub + sw],
                                start=True, stop=False)
                            for k in range(sw // 128):
                                sic = ti + (sub + k * 128) // 128
                                nc.tensor.matmul(
                                    pqk[:, sub + k * 128:sub + (k + 1) * 128],
                                    srel16_2[:, c["idx"],
                                             OFFS[sic] + 128 * ti:
                                             OFFS[sic] + 128 * ti + 128],
                                    ident16[:],
                                    start=False, stop=(k == sw // 128 - 1))
                        nc.scalar.activation(
                            c["A16T"][:, OT[ti]:OT[ti] + w],
                            pqk[:, :w],
                            mybir.ActivationFunctionType.Exp, scale=0.125)

                # --- AV + normalize per head, per s-half ---
                for c in ctxs:
                    h, jr = c["h"], c["jr"]
                    odd_tmp = None
                    if h % 2 == 1:
                        odd_tmp = work.tile([64, S], f16, tag="odd_tmp",
                                            name=f"ot{h}", bufs=2)
                    for half in range(2):
                        sh0 = half * 512
                        pav = ps_av.tile([65, 512], f32, tag="av",
                                         name=f"pav{h}_{half}", bufs=2)
                        mms = []
                        for ti in range(NT):
                            lo = max(sh0, 128 * ti)
                            hi = sh0 + 512
                            if lo >= hi:
                                continue
                            mms.append((ti, lo, hi))
                        for n, (ti, lo, hi) in enumerate(mms):
                            v16s = v16[ti][:, h * 65:(h + 1) * 65]
                            nc.tensor.matmul(
                                pav[:, lo - sh0:hi - sh0], v16s,
                                c["A16T"][:, OT[ti] + lo - 128 * ti:
                                          OT[ti] + hi - 128 * ti],
                                start=(n == 0), stop=(n == len(mms) - 1))
                        rZ_row = work.tile([1, 512], f32r, tag="rZ_row",
                                           name=f"rZ{h}_{half}", bufs=2)
                        with nc.allow_low_precision(reason="f32r==f32 bits"):
                            nc.vector.reciprocal(rZ_row[:], pav[64:65, :])
                        przi = ps_qk.tile([64, 512], f32, tag="qk",
                                          name=f"prz{h}_{half}", bufs=2)
                        nc.tensor.matmul(przi[:], ones1[:], rZ_row[:],
                                         start=True, stop=True)
                        rzb = work.tile([64, 512], f32, tag="rzb_sb", bufs=2,
                                        name=f"rzb{h}_{half}")
                        if half == 0:
                            nc.vector.tensor_copy(rzb[:], przi[:])
                        else:
                            nc.scalar.copy(rzb[:], przi[:])
                        if h % 2 == 0:
                            nc.vector.tensor_mul(
                                attn_outT[jb][0:64, sh0:sh0 + 512],
                                pav[0:64, :], rzb[:])
                        else:
                            nc.vector.tensor_mul(odd_tmp[:, sh0:sh0 + 512],
                                                 pav[0:64, :], rzb[:])
                    if h % 2 == 1:
                        nc.sync.dma_start(out=attn_outT[jb][64:128, :],
                                            in_=odd_tmp[:])

        # ---- output projection (natural layout, direct) ----
        with tc.tile_pool(name="ps_o", bufs=2, space="PSUM") as ps_o, \
             tc.tile_pool(name="stage_o", bufs=2) as stg:
            for sc in range(NT):
                po1 = ps_o.tile([128, 512], f32, tag="o1")
                po2 = ps_o.tile([128, 256], f32, tag="o2")
                for ib in range(NI):
                    lhs = attn_outT[ib][:, sc * 128:(sc + 1) * 128]
                    nc.tensor.matmul(po1[:], lhs, woT[ib][:, 0:512],
                                     start=(ib == 0), stop=(ib == NI - 1))
                    nc.tensor.matmul(po2[:], lhs, woT[ib][:, 512:768],
                                     start=(ib == 0), stop=(ib == NI - 1))
                outs = stg.tile([128, D], f32, tag="out_strip")
                nc.vector.tensor_add(outs[:, 0:512], po1[:],
                                     bo_row[:, 0:512])
                nc.vector.tensor_add(outs[:, 512:768], po2[:],
                                     bo_row[:, 512:768])
                nc.sync.dma_start(out=out.ap()[sc * 128:(sc + 1) * 128, :],
                                  in_=outs[:])


_NC = None


def kernel(**inputs):
    global _NC
    if _NC is None:
        _NC = build_nc()
    Q = np.ascontiguousarray(np.asarray(inputs["Q"], dtype=np.float32))
    K = np.ascontiguousarray(np.asarray(inputs["K"], dtype=np.float32))
    V = np.ascontiguousarray(np.asarray(inputs["V"], dtype=np.float32))
    shared = {
        name: np.ascontiguousarray(np.asarray(inputs[name], dtype=np.float32))
        for name in ("Wq", "Wk", "Wv", "Wo", "bq", "bk", "bv", "bo", "Er")
    }
    in_maps = [
        {"Qb": Q[c], "Kb": K[c], "Vb": V[c], **shared} for c in range(N_CORES)
    ]
    global _last_in_maps
    _last_in_maps = in_maps
    res = run_bass_kernel_spmd(_NC, in_maps, list(range(N_CORES)))
    return np.stack([res.results[c]["out"] for c in range(N_CORES)], axis=0)
